# revision 12
# baseline (speedup 1.0000x reference)
"""Bass/Trainium2 kernel for nn_Discriminator (scalar-input LSTM + score head).

Computation per batch element b (B=262144, T=20, H=10):
    h0 = c0 = 0
    for t in 0..T-1:
        s[b,t]  = sigmoid(leaky_relu(h @ W1.T + b1, 0.2) @ W2.T + b2)   # h BEFORE update
        gates   = x_t * w_x + h @ W_hh.T + (b_ih + b_hh)
        i,f,g,o = split(gates, 4)
        c       = sig(f)*c + sig(i)*tanh(g)
        h       = sig(o)*tanh(c)
Returns (scores [B,T], masks [B,T]) — masks is a pure passthrough (host side).

Sharding: pure data parallel over batch across 8 NeuronCores. Batch padded
262144 -> 270336 = 8*33792; per core 33792 = G(4) * C(11 chunks) * BF(768).

Per-core layout: C=11 batch-chunks of BF=768 columns stacked on SBUF partitions.
A per-group mega tile M[128, T*BF] (bf16) holds the recurrent state h for every
step:
  partitions   0:110 = h (chunk q at rows 10q..10q+9); column slice t = h_t
  partitions 110:121 = x_t per chunk (row 110+q), pre-packed on host
  partition      121 = constant 1.0 (folds gate/L1 biases into the matmuls)
  partitions 122:128 = 0 (pad so gate matmuls contract over a full 128)
Gates: one block-diagonal bf16 matmul per gate into its own 1-bank PSUM tile
(separate tags, bufs=2 — keeps per-matmul semaphore fan-in low). Activations on
ScalarE (single 'sigmoid_and_others' table set: Sigmoid/Tanh/Prelu), c/h update
on VectorE in bf16 (2x DVE mode). Phase 2 (score head) re-reads M after the
recurrence, so it overlaps the next group's recurrence on other engines.
"""

import numpy as np
import ml_dtypes

B_FULL = 262144
T = 20
H = 10
N_CORES = 8
C = 11          # batch chunks per group stacked on partitions
BF = 512        # free-dim (batch columns) per chunk
G = 6           # groups per core
B_CORE = C * BF * G          # 33792
B_PAD = B_CORE * N_CORES     # 270336
COLS = T * BF                # columns in the per-group mega tile
XROWS = 18                   # rows 0:11 x, 11 ones, 12:18 zeros -> M[110:128]

BF16 = ml_dtypes.bfloat16

_CACHE = {}


def _build_bass():
    import concourse.bacc as bacc
    import concourse.tile as tile
    import concourse.mybir as mybir

    f32 = mybir.dt.float32
    b16 = mybir.dt.bfloat16
    AF = mybir.ActivationFunctionType

    nc = bacc.Bacc("TRN2")
    xin = nc.dram_tensor("xin", [XROWS, G * COLS], b16, kind="ExternalInput")
    wcat = nc.dram_tensor("wcat", [128, 506], b16, kind="ExternalInput")
    b2v = nc.dram_tensor("b2v", [C, 1], f32, kind="ExternalInput")
    sout = nc.dram_tensor("sout", [C, G * COLS], f32, kind="ExternalOutput")

    with tile.TileContext(nc) as tc:
        with (
            tc.tile_pool(name="singles", bufs=1) as singles,
            tc.tile_pool(name="mpool", bufs=1) as mpool,
            tc.tile_pool(name="cpool", bufs=1) as cpool,
            tc.tile_pool(name="temps", bufs=4) as temps,
            tc.tile_pool(name="psum", bufs=2, space="PSUM") as ps,
        ):
            wsb = singles.tile([128, 506], b16)
            nc.sync.dma_start(out=wsb, in_=wcat[:, :])
            b2sb = singles.tile([C, 1], f32)
            nc.sync.dma_start(out=b2sb, in_=b2v[:, :])

            Ms = []
            cts = []
            for g in range(G):
                M = mpool.tile([128, COLS], b16, tag=f"m{g}")
                nc.sync.dma_start(
                    out=M[110:128, :], in_=xin[:, g * COLS:(g + 1) * COLS]
                )
                nc.vector.memset(M[0:110, 0:BF], 0.0)
                c_t = cpool.tile([110, BF], b16, tag=f"c{g}")
                nc.vector.memset(c_t, 0.0)
                Ms.append(M)
                cts.append(c_t)

            # ---- phase 1: LSTM recurrence, groups interleaved so two
            # independent chains keep every engine busy ----
            for t in range(T):
                col = t * BF
                for g in range(G):
                    M, c_t = Ms[g], cts[g]
                    rhs = M[:, col:col + BF]
                    pgif = ps.tile([110, 2 * BF], f32, tag="pgif")
                    nc.tensor.matmul(
                        out=pgif[:, 0:BF], lhsT=wsb[:, 0:110], rhs=rhs,
                        start=True, stop=True,
                    )
                    nc.tensor.matmul(
                        out=pgif[:, BF:2 * BF], lhsT=wsb[:, 110:220], rhs=rhs,
                        start=True, stop=True,
                    )
                    pgg = ps.tile([110, BF], f32, tag="pgg")
                    nc.tensor.matmul(
                        out=pgg, lhsT=wsb[:, 220:330], rhs=rhs,
                        start=True, stop=True,
                    )
                    pgo = ps.tile([110, BF], f32, tag="pgo")
                    nc.tensor.matmul(
                        out=pgo, lhsT=wsb[:, 330:440], rhs=rhs,
                        start=True, stop=True,
                    )
                    sif = temps.tile([110, 2 * BF], b16)
                    nc.scalar.activation(sif, pgif, AF.Sigmoid)
                    tg = temps.tile([110, BF], b16)
                    nc.scalar.activation(tg, pgg, AF.Tanh)
                    so = temps.tile([110, BF], b16)
                    nc.scalar.activation(so, pgo, AF.Sigmoid)
                    t2 = temps.tile([110, BF], b16)
                    nc.vector.tensor_mul(t2, sif[:, 0:BF], tg)
                    t1 = temps.tile([110, BF], b16)
                    nc.vector.tensor_mul(t1, sif[:, BF:2 * BF], c_t)
                    nc.vector.tensor_add(c_t, t1, t2)
                    tnc = temps.tile([110, BF], b16)
                    nc.scalar.activation(tnc, c_t, AF.Tanh)
                    if t < T - 1:
                        nc.vector.tensor_mul(
                            M[0:110, col + BF:col + 2 * BF], so, tnc
                        )

            # ---- phase 2: score head over all stored h_t ----
            for t in range(T):
                col = t * BF
                for g in range(G):
                    M = Ms[g]
                    pl1 = ps.tile([55, BF], f32, tag="pgg")
                    nc.tensor.matmul(
                        out=pl1,
                        lhsT=wsb[:, 440:495],
                        rhs=M[:, col:col + BF],
                        start=True,
                        stop=True,
                    )
                    lk = temps.tile([55, BF], b16)
                    nc.scalar.activation(lk, pl1, AF.Prelu, alpha=0.2)
                    pl2 = ps.tile([C, BF], f32, tag="pgo")
                    nc.tensor.matmul(
                        out=pl2,
                        lhsT=wsb[0:55, 495:506],
                        rhs=lk,
                        start=True,
                        stop=True,
                    )
                    ssb = temps.tile([C, BF], f32)
                    nc.scalar.activation(ssb, pl2, AF.Sigmoid, bias=b2sb[:, 0:1])
                    nc.sync.dma_start(
                        out=sout[:, g * COLS + col:g * COLS + col + BF], in_=ssb
                    )
    nc.compile()
    return nc


def _pack_weights(W_ih, W_hh, b_ih, b_hh, W1, b1, W2, b2):
    """Host-side lhsT construction. wcat[128, 506] (bf16):
    cols 0:440   four gate lhsT blocks [128,110] (order i,f,g,o)
    cols 440:495 L1 lhsT [128,55]
    cols 495:506 L2 lhsT [55,11] (rows 0:55)
    """
    w = np.zeros((128, 506), np.float32)
    wx = W_ih[:, 0]
    b = b_ih + b_hh
    for gi in range(4):
        base = gi * 110
        for q in range(C):
            for j in range(H):
                colj = base + q * H + j
                w[q * H:(q + 1) * H, colj] = W_hh[gi * H + j, :]
                w[110 + q, colj] = wx[gi * H + j]
                w[121, colj] = b[gi * H + j]
    for q in range(C):
        for m in range(5):
            colj = 440 + q * 5 + m
            w[q * H:(q + 1) * H, colj] = W1[m, :]
            w[121, colj] = b1[m]
    for q in range(C):
        w[5 * q:5 * q + 5, 495 + q] = W2[0, :]
    return w.astype(BF16)


def _pack_xin(vals_core):
    """vals_core [B_CORE, T] -> xin [XROWS, G*COLS] (bf16).
    xin[q, g*COLS + t*BF + j] = vals_core[g*C*BF + q*BF + j, t]"""
    v = vals_core.reshape(G, C, BF, T)            # (g, q, j, t)
    x = np.transpose(v, (1, 0, 3, 2))             # (q, g, t, j)
    x = x.reshape(C, G * COLS)
    out = np.zeros((XROWS, G * COLS), BF16)
    out[0:C] = x.astype(BF16)
    out[C] = BF16(1.0)
    return out


def _unpack_scores(sout_core):
    """inverse of _pack_xin for the scores"""
    s = sout_core.reshape(C, G, T, BF)            # (q, g, t, j)
    s = np.transpose(s, (1, 0, 3, 2))             # (g, q, j, t)
    return s.reshape(B_CORE, T)


def kernel(values, masks, W_ih, W_hh, b_ih, b_hh, W1, b1, W2, b2, args, direct,
           _trace=False):
    values = np.asarray(values, np.float32)
    masks = np.asarray(masks, np.float32)
    W_ih = np.asarray(W_ih, np.float32)
    W_hh = np.asarray(W_hh, np.float32)
    b_ih = np.asarray(b_ih, np.float32)
    b_hh = np.asarray(b_hh, np.float32)
    W1 = np.asarray(W1, np.float32)
    b1 = np.asarray(b1, np.float32)
    W2 = np.asarray(W2, np.float32)
    b2 = np.asarray(b2, np.float32)

    d = int(direct)
    vs = values if d == 0 else values[:, ::-1]
    ms = masks if d == 0 else masks[:, ::-1]

    if "nc" not in _CACHE:
        _CACHE["nc"] = _build_bass()
    nc = _CACHE["nc"]

    wcat = _pack_weights(W_ih, W_hh, b_ih, b_hh, W1, b1, W2, b2)
    b2arr = np.full((C, 1), float(b2[0]), np.float32)

    vpad = np.zeros((B_PAD, T), np.float32)
    vpad[:B_FULL] = vs
    in_maps = []
    for core in range(N_CORES):
        sl = vpad[core * B_CORE:(core + 1) * B_CORE]
        in_maps.append({"xin": _pack_xin(sl), "wcat": wcat, "b2v": b2arr})

    from concourse.bass_utils import run_bass_kernel_spmd

    res = run_bass_kernel_spmd(
        nc, in_maps, core_ids=list(range(N_CORES)), trace=_trace
    )
    scores = np.concatenate(
        [_unpack_scores(r["sout"]) for r in res.results], axis=0
    )[:B_FULL]
    if _trace:
        return (scores, ms), res
    return scores, ms
